# revision 4
# baseline (speedup 1.0000x reference)
"""Trainium2 Bass kernel for nn_EngramModule (embedding_lookup).

Sharding: 8 cores; core c handles batch c//2, sequence half c%2 (4096 output
tokens per core). Each core computes 4224 striped positions: local position
ell = 33*p + j (p = SBUF partition, j = column), covering seq range
[s0-2, s0-2+4224) — a 2-token left halo for the causal conv plus tail padding.

Pipeline per core (all compute on device):
  1. hash: digit-plane term tables (built host-side from compile-time hash
     constants), gathered by raw ids via dma_gather; XOR + digit-sum +
     conditional-subtract mod 1023 on DVE (exact in fp32/bitwise domains).
  2. fused embedding table [8192, 128] fp16, gathered TRANSPOSED via
     dma_gather(transpose=True) -> memT per head [96(+pad), 4224].
  3. fp16 matmuls (K=96 per head, 8-chunk PSUM accumulation) for key/value
     projections; rmsnorm via ACT Square+accum; gate dot via DVE
     scalar_tensor_tensor accum; sigmoid/sqrt on ACT.
  4. causal depthwise conv along j (free dim) with a partition-shift halo.

Host/wire strategy (the axon tunnel moves ~60-90 MB/s half-duplex, with
~60-100 ms fixed cost per transfer, so bytes-on-wire and transfer count
dominate wall time):
  - hash digit tables and per-core position masks are compile-time constants
    kept device-resident across calls (zero steady-state wire cost);
  - all per-core runtime inputs (hidden fp16 + id streams) are packed into a
    single flat f16 tensor "rt16" -> one h2d transfer;
  - the shared tensors (fused embedding, projections, conv weight) are packed
    into one flat f16 tensor "sh16", uploaded sharded (1/8 per core) and
    replicated on-device by a tiny resharding jit -> 4.5 MB on the wire
    instead of 35.6 MB;
  - the output comes back fp16;
  - the NEFF's output operand is a persistent non-donated device zeros buffer
    (the kernel writes every output element);
  - one jitted shard_map executable cached across calls.
"""

import sys
import numpy as np

sys.path.insert(0, "/opt/trn_rl_repo")

from contextlib import ExitStack

import concourse.bass as bass
import concourse.bacc as bacc
import concourse.tile as tile
from concourse import mybir

F32 = mybir.dt.float32
F16 = mybir.dt.float16
I32 = mybir.dt.int32
I16 = mybir.dt.int16
AOT = mybir.AluOpType
AFT = mybir.ActivationFunctionType

# --- problem constants (mirrors reference.py) ---
LAYER_ID = 0
HASH_SEED = 17
N_GRAM_LIST = [2, 3]
NUM_HEADS = 4
HASH_MODULUS = 1023
HIDDEN = 768
HEAD_DIM = 96
CONV_K = 3
EPS = 1e-6
B, S = 4, 8192
VOCAB = 10240

# --- sharding/layout constants ---
NC = 8           # cores
P = 128          # partitions
TB = 33          # tokens per partition (columns)
TC = P * TB      # 4224 computed positions per core
TOUT = 4096      # output tokens per core
NSLOT = 8        # 4 heads x 2 n-grams
NDIG = 5         # 10-bit digit planes covering 50 bits
TABW = 64        # padded table row width (ints) -> 256B rows for dma_gather

# --- packed input layouts (element offsets into flat f16 tensors) ---
NIDS = P * (TC // 16)                 # 33792 i16 per id stream
RT_HID = 0                            # hidden fp16 [TC*HIDDEN]
RT_IDS = TC * HIDDEN                  # ids0/1/2 i16 (bitcast), each NIDS
NRT = RT_IDS + 3 * NIDS               # rt16 total elements

SH_FEMB = 0                           # femb f16 [NSLOT*1024*P]
SH_WK = SH_FEMB + NSLOT * 1024 * P    # wk f16 [HEAD_DIM*NSLOT*HIDDEN]
SH_WV = SH_WK + HEAD_DIM * NSLOT * HIDDEN
SH_CW = SH_WV + HEAD_DIM * NSLOT * HIDDEN   # conv weight f16 [CONV_K*HIDDEN]
NSH = SH_CW + CONV_K * HIDDEN         # sh16 total elements (divisible by 8)


def _hash_params(n):
    max_int = (1 << 31) - 1
    mults, offs = [], []
    for h in range(NUM_HEADS):
        base = HASH_SEED + 10007 * (LAYER_ID + 1) + 1543 * (n + 1) + 8191 * (h + 1)
        row = []
        for pp in range(n):
            v = (base + 32771 * (pp + 1) + 65537 * (h + 1) * (pp + 1)) % max_int
            row.append(v * 2 + 1)
        mults.append(row)
        offs.append((base * 2147483647 + 97 * (n + h + 1)) % max_int)
    return np.array(mults, dtype=np.int64), np.array(offs, dtype=np.int64)


def _build_const_tables():
    """Host tables derived only from compile-time hash constants."""
    tabs = []        # 5 tables [VOCAB, TABW] int32: (n, pos) in order (2,0),(2,1),(3,0),(3,1),(3,2)
    offm = np.zeros(NSLOT, dtype=np.int64)   # off % 1023 per slot
    v = np.arange(VOCAB, dtype=np.int64)
    for gi, n in enumerate(N_GRAM_LIST):
        mult, off = _hash_params(n)
        for h in range(NUM_HEADS):
            offm[gi * 4 + h] = off[h] % HASH_MODULUS
        for pos in range(n):
            t = np.zeros((VOCAB, TABW), dtype=np.int32)
            for h in range(NUM_HEADS):
                u = v * mult[h][pos]        # exact int64, < 2^46
                for d in range(NDIG):
                    t[:, h * NDIG + d] = ((u >> (10 * d)) & 1023).astype(np.int32)
            tabs.append(t)
    return tabs, offm


def _wrap16(flat):
    """[TC] stream -> [128, TC//16] idx layout: (i%16, i//16), replicated 8x."""
    w = flat.reshape(TC // 16, 16).T.astype(np.int16)
    return np.ascontiguousarray(np.tile(w, (8, 1)))


_TABS, _OFFM = _build_const_tables()

# position helpers: stream n = j*128 + p holds token ell = 33*p + j
_n = np.arange(TC)
_p_of_n = _n % P
_j_of_n = _n // P
_ell_of_n = TB * _p_of_n + _j_of_n          # token index for stream position n
_pj_ell = (TB * np.arange(P)[:, None] + np.arange(TB)[None, :])  # [128, 33]


def _core_cmeta(s0):
    """Compile-time per-core metadata: mask, mb, offm in [3, 128, 264] i32."""
    g_pj = s0 - 2 + _pj_ell                   # [128, 33]
    valid = (g_pj >= 0) & (g_pj < S)
    mask = np.zeros((P, TB, NSLOT), dtype=np.int32)
    for slot in range(NSLOT):
        n = N_GRAM_LIST[slot // 4]
        mask[:, :, slot] = (valid & (g_pj >= n - 1)).astype(np.int32)
    mb = mask + 1024 * np.arange(NSLOT, dtype=np.int32)[None, None, :]
    offm = np.broadcast_to(_OFFM.astype(np.int32), (P, TB, NSLOT))
    return np.stack([mask.reshape(P, -1), mb.reshape(P, -1),
                     np.ascontiguousarray(offm.reshape(P, -1))]).astype(np.int32)


def _core_ids(ids_b, s0):
    """id streams for the 5 hash-table gathers (shifts 0,1,2): [3, 128, 264] i16."""
    ids_pad = np.zeros(S + 8, dtype=np.int64)
    ids_pad[4: 4 + S] = ids_b
    idw = []
    for d in range(3):
        g = s0 - 2 + _ell_of_n - d           # global pos of (token - d)
        vals = ids_pad[np.clip(g, -4, S - 1) + 4]
        vals = np.where((g >= 0) & (g < S), vals, 0)
        idw.append(_wrap16(vals))
    return np.stack(idw)


def _core_hidden_f16(hidden_b, s0, out):
    """Write the [TC, HIDDEN] fp16 block for one core into `out`."""
    lo, hi = max(0, -(s0 - 2)), min(TC, S - (s0 - 2))
    if lo > 0:
        out[:lo] = 0
    if hi < TC:
        out[hi:] = 0
    out[lo:hi] = hidden_b[s0 - 2 + lo: s0 - 2 + hi]


def _build_sh16(emb, w_key, w_value, key_norm_w, value_norm_w, conv_w):
    """Shared tensors packed into one flat f16 array of NSH elements."""
    sh = np.empty(NSH, dtype=np.float16)
    femb = sh[SH_FEMB:SH_WK].reshape(NSLOT * 1024, P)
    femb[:, :HEAD_DIM] = emb.reshape(NSLOT * 1024, HEAD_DIM).astype(np.float16)
    femb[:, HEAD_DIM:] = 0

    def wprep(dst, w, nw):
        wt = (w * nw[:, None]).T.astype(np.float16)      # [m, o] = w[o, m]*nw[o]
        # [96, 8*768]: col h*768+o = wt[h*96+d, o]
        out = dst.reshape(HEAD_DIM, NSLOT * HIDDEN)
        for h in range(NSLOT):
            out[:, h * HIDDEN:(h + 1) * HIDDEN] = wt[h * HEAD_DIM:(h + 1) * HEAD_DIM, :]

    wprep(sh[SH_WK:SH_WV], w_key, key_norm_w)
    wprep(sh[SH_WV:SH_CW], w_value, value_norm_w)
    sh[SH_CW:] = conv_w.T.astype(np.float16).ravel()     # [3, 768]
    return sh


def _build_nc():
    nc = bacc.Bacc("TRN2", target_bir_lowering=False, num_devices=NC)

    din = {}
    din["rt16"] = nc.dram_tensor("rt16", [NRT], F16, kind="ExternalInput")
    din["sh16"] = nc.dram_tensor("sh16", [NSH], F16, kind="ExternalInput")
    din["cmeta"] = nc.dram_tensor("cmeta", [3, P, TB * NSLOT], I32, kind="ExternalInput")
    for i in range(5):
        din[f"tab{i}"] = nc.dram_tensor(f"tab{i}", [VOCAB, TABW], I32, kind="ExternalInput")
    out_d = nc.dram_tensor("out", [TOUT, HIDDEN], F16, kind="ExternalOutput")
    fidx_stage = nc.dram_tensor("fidx_stage", [NSLOT, P, TB], I16)  # internal DRAM

    with tile.TileContext(nc) as tc:
        with ExitStack() as ctx:
            _emit(ctx, tc, nc, din, out_d, fidx_stage)
    nc.compile()
    return nc


def _emit(ctx, tc, nc, din, out_d, fidx_stage):
    consts = ctx.enter_context(tc.tile_pool(name="consts", bufs=1))
    w16p = ctx.enter_context(tc.tile_pool(name="w16p", bufs=1))
    work = ctx.enter_context(tc.tile_pool(name="work", bufs=2))
    small = ctx.enter_context(tc.tile_pool(name="small", bufs=4))
    gpool = ctx.enter_context(tc.tile_pool(name="gpool", bufs=6))
    psk = ctx.enter_context(tc.tile_pool(name="psk", bufs=1, space="PSUM"))
    psv = ctx.enter_context(tc.tile_pool(name="psv", bufs=3, space="PSUM"))

    # packed input views
    femb_ap = din["sh16"][SH_FEMB:SH_WK].rearrange("(v p) -> v p", p=P)
    wk_ap = din["sh16"][SH_WK:SH_WV].rearrange("(d x) -> d x", d=HEAD_DIM)
    wv_ap = din["sh16"][SH_WV:SH_CW].rearrange("(d x) -> d x", d=HEAD_DIM)
    cw_ap = din["sh16"][SH_CW:NSH].rearrange("(k h) -> k h", k=CONV_K)
    hidv = din["rt16"][RT_HID:RT_IDS].rearrange("(p x) -> p x", p=P)  # [128, 33*768]
    ids_ap = [
        din["rt16"][RT_IDS + i * NIDS: RT_IDS + (i + 1) * NIDS]
        .bitcast(I16).rearrange("(p x) -> p x", p=P)
        for i in range(3)]

    # ---- constants into SBUF ----
    wk_sb = consts.tile([HEAD_DIM, NSLOT * HIDDEN], F16, tag="wk")
    nc.sync.dma_start(out=wk_sb[:], in_=wk_ap)
    wv_sb = consts.tile([HEAD_DIM, NSLOT * HIDDEN], F16, tag="wv")
    nc.sync.dma_start(out=wv_sb[:], in_=wv_ap)
    cwb = []
    for k in range(CONV_K):
        t16 = consts.tile([P, HIDDEN], F16, tag=f"cw16_{k}")
        row = cw_ap[k]
        bcast = bass.AP(tensor=row.tensor, offset=row.offset, ap=[[0, P]] + list(row.ap))
        nc.sync.dma_start(out=t16[:], in_=bcast)
        t = consts.tile([P, HIDDEN], F32, tag=f"cw{k}")
        nc.vector.tensor_copy(out=t[:], in_=t16[:])
        cwb.append(t)
    meta = []
    for i in range(3):
        t = consts.tile([P, TB * NSLOT], I32, tag=f"meta{i}")
        nc.sync.dma_start(out=t[:], in_=din["cmeta"][i])
        meta.append(t)
    mask_t, mb_t, offm_t = meta
    idt = []
    for i in range(3):
        t = consts.tile([P, TC // 16], I16, tag=f"ids{i}")
        nc.sync.dma_start(out=t[:], in_=ids_ap[i])
        idt.append(t)

    # ---- phase 1: hash (transient pool, closed before memT allocation) ----
    hashp_cm = tc.tile_pool(name="hashp", bufs=1)
    hashp = hashp_cm.__enter__()
    # 5 table gathers; window pos p of n-gram n uses id shift (n-1-p)
    gshift = [(0, 1), (0, 0), (1, 2), (1, 1), (1, 0)]  # (group, shift) per tab
    gtiles = []
    for i, (gi, sh) in enumerate(gshift):
        g = hashp.tile([P, TB * TABW], I32, tag=f"g{i}")
        nc.gpsimd.dma_gather(
            out_ap=g[:].rearrange("p (a b) -> p a b", b=TABW),
            in_ap=din[f"tab{i}"][:], idxs_ap=idt[sh][:],
            num_idxs=TC, num_idxs_reg=TC, elem_size=TABW,
            single_packet=False)
        gtiles.append(g)

    fidx = hashp.tile([P, TB * NSLOT], I32, tag="fidx")
    for gi, tabs in ((0, (0, 1)), (1, (2, 3, 4))):
        x = hashp.tile([P, TB, 4, NDIG], I32, tag=f"x{gi}")
        g0 = gtiles[tabs[0]][:].rearrange("p (t w) -> p t w", w=TABW)[:, :, 0:20]
        g0 = g0.rearrange("p t (h d) -> p t h d", d=NDIG)
        g1 = gtiles[tabs[1]][:].rearrange("p (t w) -> p t w", w=TABW)[:, :, 0:20]
        g1 = g1.rearrange("p t (h d) -> p t h d", d=NDIG)
        nc.vector.tensor_tensor(out=x[:], in0=g0, in1=g1, op=AOT.bitwise_xor)
        if len(tabs) == 3:
            g2 = gtiles[tabs[2]][:].rearrange("p (t w) -> p t w", w=TABW)[:, :, 0:20]
            g2 = g2.rearrange("p t (h d) -> p t h d", d=NDIG)
            nc.vector.tensor_tensor(out=x[:], in0=x[:], in1=g2, op=AOT.bitwise_xor)
        # digit sum -> V [128, 33, 4]
        v = hashp.tile([P, TB, 4], I32, tag=f"v{gi}")
        t1 = hashp.tile([P, TB, 4], I32, tag=f"t1{gi}")
        nc.vector.tensor_tensor(out=v[:], in0=x[:, :, :, 0], in1=x[:, :, :, 1], op=AOT.add)
        nc.vector.tensor_tensor(out=t1[:], in0=x[:, :, :, 2], in1=x[:, :, :, 3], op=AOT.add)
        nc.vector.tensor_tensor(out=v[:], in0=v[:], in1=t1[:], op=AOT.add)
        nc.vector.tensor_tensor(out=v[:], in0=v[:], in1=x[:, :, :, 4], op=AOT.add)
        om = offm_t[:].rearrange("p (t s) -> p t s", s=NSLOT)[:, :, gi * 4:(gi + 1) * 4]
        nc.vector.tensor_tensor(out=v[:], in0=v[:], in1=om, op=AOT.add)
        # mod 1023 via conditional subtracts
        for thr in (4092, 2046, 1023):
            nc.vector.tensor_single_scalar(out=t1[:], in_=v[:], scalar=float(thr), op=AOT.is_ge)
            nc.vector.tensor_scalar_mul(t1[:], t1[:], float(thr))
            nc.vector.tensor_tensor(out=v[:], in0=v[:], in1=t1[:], op=AOT.subtract)
        # fidx slots = V*mask + mb
        msk = mask_t[:].rearrange("p (t s) -> p t s", s=NSLOT)[:, :, gi * 4:(gi + 1) * 4]
        mbs = mb_t[:].rearrange("p (t s) -> p t s", s=NSLOT)[:, :, gi * 4:(gi + 1) * 4]
        nc.vector.tensor_tensor(out=v[:], in0=v[:], in1=msk, op=AOT.mult)
        fslots = fidx[:].rearrange("p (t s) -> p t s", s=NSLOT)[:, :, gi * 4:(gi + 1) * 4]
        nc.vector.tensor_tensor(out=fslots, in0=v[:], in1=mbs, op=AOT.add)

    # ---- fidx -> wrapped int16 idx tiles (per head) via DRAM staging ----
    w16 = []
    for h in range(NSLOT):
        c16 = hashp.tile([P, TB], I16, tag=f"c16_{h}")
        nc.vector.tensor_copy(
            out=c16[:], in_=fidx[:].rearrange("p (t s) -> p t s", s=NSLOT)[:, :, h])
        nc.sync.dma_start(out=fidx_stage[h], in_=c16[:])   # [128, 33] -> DRAM
        # wrap: w16s[c, j*8+q] = stage[q*16+c, j]
        w16s = hashp.tile([16, TC // 16], I16, tag=f"w16s_{h}")
        src = bass.AP(
            tensor=fidx_stage.handle if hasattr(fidx_stage, "handle") else fidx_stage,
            offset=h * P * TB,
            ap=[[TB, 16], [1, TB], [16 * TB, 8]])   # (c, j, q) iteration
        dst = w16s[:].rearrange("c (j q) -> c j q", q=8)
        nc.sync.dma_start(out=dst, in_=src)
        wt = w16p.tile([P, TC // 16], I16, tag=f"w16_{h}")
        nc.sync.dma_start(out=wt[0:16, :], in_=w16s[:])
        for blk in (16, 32, 64):
            nc.sync.dma_start(out=wt[blk:2 * blk, :], in_=wt[0:blk, :])
        w16.append(wt)

    hashp_cm.__exit__(None, None, None)

    # ---- phase 2: transposed fp16 embedding gathers ----
    memp = ctx.enter_context(tc.tile_pool(name="memp", bufs=1))
    memT = []
    for h in range(NSLOT):
        m = memp.tile([P, TC], F16, tag=f"memT{h}")
        nc.gpsimd.dma_gather(
            out_ap=m[:].rearrange("p (a b) -> p a b", b=TC),
            in_ap=femb_ap, idxs_ap=w16[h][:],
            num_idxs=TC, num_idxs_reg=TC, elem_size=P, transpose=True,
            single_packet=False)
        memT.append(m)

    # ---- phase 3: column loop ----
    # gcols[m] holds gated values at ell = 33p + m - 2. m<4 pinned (late conv
    # cols 0/1 + halo); m>=4 rolling 6-slot window.
    gcols = {}
    for m in range(4):
        gcols[m] = consts.tile([P, HIDDEN], F32, tag=f"gcpin{m}", name=f"gcpin{m}")
    nc.vector.memset(gcols[0][:], 0.0)
    nc.vector.memset(gcols[1][:], 0.0)


    def value_col(j):
        if j + 2 >= 4:
            gcols[j + 2] = gpool.tile([P, HIDDEN], F32, tag="gcroll", name="gcroll")
        hid_j = work.tile([P, HIDDEN], F16, tag="hid")
        nc.sync.dma_start(out=hid_j[:], in_=hidv[:, j * HIDDEN:(j + 1) * HIDDEN])
        pk = psk.tile([P, HIDDEN], F32, tag="pk")
        pv = psv.tile([P, HIDDEN], F32, tag="pv")
        for ps, wsb in ((pk, wk_sb), (pv, wv_sb)):
            for h in range(NSLOT):
                lhs = memT[h][0:HEAD_DIM, j * P:(j + 1) * P]
                nc.tensor.matmul(out=ps[:, 0:512],
                                 lhsT=lhs, rhs=wsb[:, h * HIDDEN: h * HIDDEN + 512],
                                 start=(h == 0), stop=(h == NSLOT - 1))
                nc.tensor.matmul(out=ps[:, 512:HIDDEN],
                                 lhsT=lhs, rhs=wsb[:, h * HIDDEN + 512:(h + 1) * HIDDEN],
                                 start=(h == 0), stop=(h == NSLOT - 1))
        scr = work.tile([P, HIDDEN], F32, tag="scr")
        ssq_k = small.tile([P, 1], F32, tag="ssqk")
        nc.scalar.activation(out=scr[:], in_=pk[:], func=AFT.Square, accum_out=ssq_k[:])
        scr2 = work.tile([P, HIDDEN], F32, tag="scr2")
        dot = small.tile([P, 1], F32, tag="dot")
        nc.vector.scalar_tensor_tensor(
            out=scr2[:], in0=hid_j[:], scalar=1.0, in1=pk[:],
            op0=AOT.mult, op1=AOT.mult, accum_out=dot[:])
        scr3 = work.tile([P, HIDDEN], F32, tag="scr3")
        ssq_v = small.tile([P, 1], F32, tag="ssqv")
        nc.scalar.activation(out=scr3[:], in_=pv[:], func=AFT.Square, accum_out=ssq_v[:])

        rk = small.tile([P, 1], F32, tag="rk")
        nc.vector.tensor_scalar_add(rk[:], ssq_k[:], float(HIDDEN) * EPS)
        nc.vector.reciprocal(rk[:], rk[:])
        nc.scalar.activation(out=rk[:], in_=rk[:], func=AFT.Sqrt)
        gate = small.tile([P, 1], F32, tag="gate")
        nc.scalar.activation(out=gate[:], in_=dot[:], func=AFT.Sigmoid, scale=rk[:])
        rv = small.tile([P, 1], F32, tag="rv")
        nc.vector.tensor_scalar_add(rv[:], ssq_v[:], float(HIDDEN) * EPS)
        nc.vector.reciprocal(rv[:], rv[:])
        nc.scalar.activation(out=rv[:], in_=rv[:], func=AFT.Sqrt, scale=float(HIDDEN))
        gv = small.tile([P, 1], F32, tag="gv")
        nc.vector.tensor_mul(gv[:], gate[:], rv[:])
        nc.scalar.activation(out=gcols[j + 2][:], in_=pv[:], func=AFT.Copy, scale=gv[:])

    def conv_col(jc):
        a = work.tile([P, HIDDEN], F32, tag="cva")
        b = work.tile([P, HIDDEN], F32, tag="cvb")
        c = work.tile([P, HIDDEN], F32, tag="cvc")
        o16 = work.tile([P, HIDDEN], F16, tag="cvo")
        nc.vector.tensor_mul(a[:], gcols[jc][:], cwb[0][:])
        nc.vector.tensor_mul(b[:], gcols[jc + 1][:], cwb[1][:])
        nc.vector.tensor_mul(c[:], gcols[jc + 2][:], cwb[2][:])
        nc.gpsimd.tensor_add(a[:], a[:], b[:])
        nc.vector.tensor_tensor(out=o16[:], in0=a[:], in1=c[:], op=AOT.add)
        p0 = 1 if jc < 2 else 0
        pmax = (4095 - (jc - 2)) // TB
        np_rows = pmax - p0 + 1
        dst = bass.AP(tensor=out_d, offset=(TB * p0 + jc - 2) * HIDDEN,
                      ap=[[TB * HIDDEN, np_rows], [1, HIDDEN]])
        nc.sync.dma_start(out=dst, in_=o16[p0:pmax + 1, :])

    for j in range(TB):
        value_col(j)
        if j >= 2:
            conv_col(j - 2)
    # halo columns from partition p-1's last two value columns
    nc.sync.dma_start(out=gcols[0][1:P, :], in_=gcols[TB][0:P - 1, :])
    nc.sync.dma_start(out=gcols[1][1:P, :], in_=gcols[TB + 1][0:P - 1, :])
    conv_col(TB - 2)
    conv_col(TB - 1)
    conv_col(0)
    conv_col(1)


# ---------------------------------------------------------------------------
# Persistent runner: one jitted shard_map executable + device-resident
# constant inputs, reused across kernel() calls.
# ---------------------------------------------------------------------------

_RUNNER = None


class _Runner:
    def __init__(self):
        import jax
        from jax.sharding import Mesh, PartitionSpec, NamedSharding
        import warnings
        with warnings.catch_warnings():
            warnings.simplefilter("ignore")
            from jax.experimental.shard_map import shard_map
        from concourse.bass2jax import (
            _bass_exec_p, partition_id_tensor, install_neuronx_cc_hook)

        self.jax = jax
        nc = _build_nc()
        self.nc = nc
        install_neuronx_cc_hook()

        partition_name = nc.partition_id_tensor.name if nc.partition_id_tensor else None
        in_names, out_names, out_avals = [], [], []
        for alloc in nc.m.functions[0].allocations:
            if not isinstance(alloc, mybir.MemoryLocationSet):
                continue
            name = alloc.memorylocations[0].name
            if alloc.kind == "ExternalInput":
                if name != partition_name:
                    in_names.append(name)
            elif alloc.kind == "ExternalOutput":
                out_names.append(name)
                out_avals.append(jax.core.ShapedArray(
                    tuple(alloc.tensor_shape), mybir.dt.np(alloc.dtype)))
        self.in_names = in_names
        self.out_names = out_names
        in_names_all = in_names + out_names
        if partition_name is not None:
            in_names_all.append(partition_name)

        def _body(*args):
            operands = list(args)
            if partition_name is not None:
                operands.append(partition_id_tensor())
            outs = _bass_exec_p.bind(
                *operands, out_avals=tuple(out_avals),
                in_names=tuple(in_names_all), out_names=tuple(out_names),
                lowering_input_output_aliases=(),
                sim_require_finite=True, sim_require_nnan=True, nc=nc)
            return tuple(outs)

        devices = jax.devices()[:NC]
        mesh = Mesh(np.asarray(devices), ("core",))
        self.sh = NamedSharding(mesh, PartitionSpec("core"))
        self.rep = NamedSharding(mesh, PartitionSpec())
        # sh16 is replicated (broadcast on device); everything else sharded
        in_specs = tuple(
            PartitionSpec() if n == "sh16" else PartitionSpec("core")
            for n in in_names) + (PartitionSpec("core"),) * len(out_names)
        self.jitted = jax.jit(
            shard_map(_body, mesh=mesh, in_specs=in_specs,
                      out_specs=(PartitionSpec("core"),) * len(out_names),
                      check_rep=False),
            keep_unused=True)

        # broadcast jit: [8, NSH//8] sharded -> [NSH] replicated, on device
        self.bcast = jax.jit(lambda v: v.reshape(NSH),
                             in_shardings=self.sh, out_shardings=self.rep)

        # --- device-resident constant inputs (uploaded once) ---
        self.const_dev = {}
        for name in ("tab0", "tab1", "tab2", "tab3", "tab4"):
            i = int(name[3])
            arr = np.concatenate([_TABS[i]] * NC, axis=0)
            self.const_dev[name] = jax.device_put(arr, self.sh)
        cmeta = np.concatenate([_core_cmeta((c % 2) * TOUT) for c in range(NC)], axis=0)
        self.const_dev["cmeta"] = jax.device_put(cmeta, self.sh)

        # persistent (non-donated) output operand buffers: the kernel writes
        # every output element, so all-zeros content is never observed.
        self.zero_dev = [
            jax.device_put(np.zeros((NC * a.shape[0], *a.shape[1:]), a.dtype), self.sh)
            for a in out_avals]
        jax.block_until_ready(list(self.const_dev.values()) + self.zero_dev)

    def run(self, rt16, sh16):
        """rt16: [NC*NRT] f16 host array; sh16: [NSH] f16 host array."""
        sh16_rep = self.bcast(sh16.reshape(NC, NSH // NC))
        args = []
        for name in self.in_names:
            if name == "rt16":
                args.append(rt16)
            elif name == "sh16":
                args.append(sh16_rep)
            else:
                args.append(self.const_dev[name])
        args.extend(self.zero_dev)
        outs = self.jitted(*args)
        return np.asarray(outs[0])


def kernel(hidden_states, input_ids, emb, w_key, w_value, key_norm_w,
           value_norm_w, conv_w):
    global _RUNNER
    if _RUNNER is None:
        _RUNNER = _Runner()
    r = _RUNNER

    hidden_states = np.asarray(hidden_states, dtype=np.float32)
    input_ids_np = np.asarray(input_ids)

    sh16 = _build_sh16(
        np.asarray(emb, dtype=np.float32), np.asarray(w_key, dtype=np.float32),
        np.asarray(w_value, dtype=np.float32),
        np.asarray(key_norm_w, dtype=np.float32),
        np.asarray(value_norm_w, dtype=np.float32),
        np.asarray(conv_w, dtype=np.float32))

    rt16 = np.empty((NC, NRT), dtype=np.float16)
    for c in range(NC):
        b, half = c // 2, c % 2
        s0 = half * TOUT
        _core_hidden_f16(hidden_states[b],
                         s0, rt16[c, RT_HID:RT_IDS].reshape(TC, HIDDEN))
        rt16[c, RT_IDS:] = _core_ids(input_ids_np[b], s0).reshape(-1).view(np.float16)

    res = r.run(rt16.reshape(NC * NRT), sh16)  # [NC*TOUT, HIDDEN] fp16
    res32 = res.astype(np.float32).reshape(NC, TOUT, HIDDEN)
    out = np.empty((B, S, HIDDEN), dtype=np.float32)
    for c in range(NC):
        b, half = c // 2, c % 2
        out[b, half * TOUT:(half + 1) * TOUT] = res32[c]
    return out
